# revision 1
# baseline (speedup 1.0000x reference)
"""Trainium2 Bass kernel for the custom transformer layer.

Sharding: 8 cores = 4 batches x 2 query-row halves. Each core computes the
full layer for 512 query rows of one batch. K/V/ptm are computed for the
whole batch on both cores sharing it (duplicated; ~14% extra tensor-engine
work, but zero cross-core communication). Inputs are rotated per-core so the
own query rows always sit at positions 0..511 -> one SPMD program for all
cores.

All large matmuls run in bf16 with fp32 PSUM accumulation.  Softmax is
computed without max-subtraction (scores are bounded by construction), and
the ptm softmax / attention-softmax normalizations are folded in via
matmul-with-ones-column tricks so no partition-dim reductions are needed.
"""

import sys

sys.path.insert(0, "/opt/trn_rl_repo")

import numpy as np
import ml_dtypes

import concourse.bass as bass
import concourse.tile as tile
from concourse import bacc, mybir
from concourse.bass_utils import run_bass_kernel_spmd
from concourse.masks import make_identity

BF16 = ml_dtypes.bfloat16
F32 = mybir.dt.float32
BF = mybir.dt.bfloat16
AF = mybir.ActivationFunctionType
ALU = mybir.AluOpType

B, S, H, NH, DH, I, C, P = 4, 1024, 1280, 20, 64, 5120, 13, 8
SQ = 512          # query rows per core
KO = H // 128     # 10 k-chunks
IC = I // 128     # 40 i-chunks
LC = SQ // 128    # 4 l-chunks
MC = S // 128     # 8 m-chunks
LN_EPS = 1e-5


def bcast_ap(src: bass.AP, parts: int) -> bass.AP:
    """Partition-stride-0 broadcast AP (for DMA sources)."""
    return bass.AP(tensor=src.tensor, offset=src.offset,
                   ap=[[0, parts]] + [list(d) for d in src.ap])


def build_nc(use_mask: bool, bias_scale: float):
    nc = bacc.Bacc("TRN2", target_bir_lowering=False, debug=False, num_devices=8)

    # ---- DRAM parameters (per-core) ----
    h_d = nc.declare_dram_parameter("h", [S, H], F32, isOutput=False)
    hres_d = nc.declare_dram_parameter("hres", [SQ, H], F32, isOutput=False)
    wqT_d = nc.declare_dram_parameter("wqT", [128, KO, H], BF, isOutput=False)
    wkT_d = nc.declare_dram_parameter("wkT", [128, KO, H], BF, isOutput=False)
    wvT_d = nc.declare_dram_parameter("wvT", [128, KO, H], BF, isOutput=False)
    wptmT_d = nc.declare_dram_parameter("wptmT", [H, P], BF, isOutput=False)
    rmat_d = nc.declare_dram_parameter("rmat", [P, P], BF, isOutput=False)
    wf1T_d = nc.declare_dram_parameter("wf1T", [IC // 2, 128, KO, 256], BF,
                                       isOutput=False)
    wf2T_d = nc.declare_dram_parameter("wf2T", [IC, 128, H], BF, isOutput=False)
    bq_d = nc.declare_dram_parameter("bq", [KO, 128], F32, isOutput=False)
    bk_d = nc.declare_dram_parameter("bk", [KO, 128], F32, isOutput=False)
    bptm_d = nc.declare_dram_parameter("bptm", [P, 1], F32, isOutput=False)
    bf1_d = nc.declare_dram_parameter("bf1", [IC, 128], F32, isOutput=False)
    lng_d = nc.declare_dram_parameter("lng", [H], F32, isOutput=False)
    lnbf_d = nc.declare_dram_parameter("lnbf", [H], F32, isOutput=False)
    mb_d = nc.declare_dram_parameter("mb", [MC, 128], F32, isOutput=False)
    out_d = nc.declare_dram_parameter("out", [SQ, H], F32, isOutput=True)

    from contextlib import ExitStack
    with tile.TileContext(nc) as tc, ExitStack() as es:
        # ---- persistent constants ----
        const = es.enter_context(tc.tile_pool(name="const", bufs=1))
        ident_b = const.tile([128, 128], BF)
        make_identity(nc, ident_b)
        ident_f = const.tile([128, 128], F32)
        make_identity(nc, ident_f)
        bq_s = const.tile([128, KO], F32)
        nc.sync.dma_start(out=bq_s, in_=bq_d.ap().rearrange("c p -> p c"))
        bk_s = const.tile([128, KO], F32)
        nc.sync.dma_start(out=bk_s, in_=bk_d.ap().rearrange("c p -> p c"))
        bf1_s = const.tile([128, IC], F32)
        nc.sync.dma_start(out=bf1_s, in_=bf1_d.ap().rearrange("c p -> p c"))
        bptm_s = const.tile([P, 1], F32)
        nc.sync.dma_start(out=bptm_s, in_=bptm_d.ap())
        mb_s = const.tile([128, MC], F32)
        nc.sync.dma_start(out=mb_s, in_=mb_d.ap().rearrange("c p -> p c"))
        lng_b = const.tile([128, H], F32)
        nc.sync.dma_start(out=lng_b, in_=bcast_ap(lng_d.ap(), 128))
        lnbf_b = const.tile([128, H], F32)
        nc.sync.dma_start(out=lnbf_b, in_=bcast_ap(lnbf_d.ap(), 128))
        eps_s = const.tile([128, 1], F32)
        nc.vector.memset(eps_s, LN_EPS)
        ones8_s = const.tile([P, 1], F32)
        nc.vector.memset(ones8_s, 1.0)
        rmat_s = const.tile([P, P], BF)
        nc.sync.dma_start(out=rmat_s, in_=rmat_d.ap())
        wptm_s = const.tile([128, KO, P], BF)
        nc.sync.dma_start(out=wptm_s,
                          in_=wptmT_d.ap().rearrange("(ko p) c -> p ko c", p=128))

        # ---- lifetime-scoped persistent activations ----
        es_w = ExitStack()        # QKV weights, prefetched from t=0
        p_w = es_w.enter_context(tc.tile_pool(name="p_w", bufs=1, side="right"))
        wq_s = p_w.tile([128, KO, H], BF)
        wk_s = p_w.tile([128, KO, H], BF)
        wv_s = p_w.tile([128, KO, H], BF)
        for wt_s, wt_d in ((wq_s, wqT_d), (wk_s, wkT_d), (wv_s, wvT_d)):
            nc.gpsimd.dma_start(out=wt_s, in_=wt_d.ap())
        es_hT = ExitStack()       # phases A..C
        p_hT = es_hT.enter_context(tc.tile_pool(name="p_hT", bufs=1, side="right"))
        hT_s = p_hT.tile([128, KO, S], BF)             # h^T, feature-major
        es_attn = ExitStack()     # phases B..D
        p_attn = es_attn.enter_context(tc.tile_pool(name="p_attn", bufs=1))
        biasT_s = p_attn.tile([128, MC, SQ], BF)       # attention bias, [m, l]
        QT_s = p_attn.tile([128, KO, SQ], BF)          # (q/8)^T
        KT_s = p_attn.tile([128, KO, S], BF)           # k^T
        # V with a ones column per head: [s-part, s-chunk, head, 64+1]
        vaug_s = p_attn.tile([128, MC, NH, DH + 1], BF)

        # ================= Phase A: h -> hT (bf16) =================
        with tc.tile_pool(name="ph_a", bufs=3) as pa, \
             tc.tile_pool(name="ph_a_ps", bufs=4, space="PSUM") as paps:
            for sc in range(MC):
                hf = pa.tile([128, H], F32, tag="hf")
                heng = nc.sync if sc % 2 == 0 else nc.scalar
                heng.dma_start(out=hf, in_=h_d.ap()[sc * 128:(sc + 1) * 128, :])
                for ko in range(KO):
                    tp = paps.tile([128, 128], F32)
                    nc.tensor.transpose(tp, hf[:, ko * 128:(ko + 1) * 128], ident_f)
                    nc.vector.tensor_copy(out=hT_s[:, ko, sc * 128:(sc + 1) * 128],
                                          in_=tp)

        # ================= Phase B: ptm -> biasT =================
        with tc.tile_pool(name="ph_b", bufs=2) as pb, \
             tc.tile_pool(name="ph_b_big", bufs=1) as pbb, \
             tc.tile_pool(name="ph_b_dram", bufs=1, space="DRAM") as pbd, \
             tc.tile_pool(name="ph_b_zps", bufs=2, space="PSUM") as pbzps, \
             tc.tile_pool(name="ph_b_ps", bufs=2, space="PSUM") as pbps:
            expT_s = pbb.tile([P, S], F32)
            rz = pb.tile([1, S], F32, tag="rz")
            for n2 in range(2):
                lp = pbps.tile([P, 512], F32, tag="logits")
                for ko in range(KO):
                    nc.tensor.matmul(lp, wptm_s[:, ko, :],
                                     hT_s[:, ko, n2 * 512:(n2 + 1) * 512],
                                     start=(ko == 0), stop=(ko == KO - 1))
                nc.scalar.activation(out=expT_s[:, n2 * 512:(n2 + 1) * 512],
                                     in_=lp, func=AF.Exp, bias=bptm_s)
            for n2 in range(2):
                zp = pbzps.tile([1, 512], F32, tag="z")
                nc.tensor.matmul(zp, ones8_s,
                                 expT_s[:, n2 * 512:(n2 + 1) * 512],
                                 start=True, stop=True)
                nc.vector.reciprocal(out=rz[:, n2 * 512:(n2 + 1) * 512], in_=zp)
            zscr = pbd.tile([1, S], F32)
            nc.sync.dma_start(out=zscr, in_=rz)
            zb = pbb.tile([P, S], F32)
            nc.sync.dma_start(out=zb, in_=bcast_ap(zscr[0, :], P))
            ptmT_s = pbb.tile([P, S], BF)
            nc.vector.tensor_mul(out=ptmT_s, in0=expT_s, in1=zb)
            gp = pbps.tile([P, 512], F32, tag="g")
            nc.tensor.matmul(gp, rmat_s, ptmT_s[:, :SQ], start=True, stop=True)
            gTs = pbb.tile([P, SQ], BF)
            nc.vector.tensor_copy(out=gTs, in_=gp)
            for mc in range(MC):
                up = pbps.tile([128, SQ], F32, tag="u")
                nc.tensor.matmul(up, ptmT_s[:, mc * 128:(mc + 1) * 128], gTs,
                                 start=True, stop=True)
                tt = pb.tile([128, SQ], F32, tag="tanh")
                nc.scalar.activation(out=tt, in_=up, func=AF.Tanh)
                if use_mask:
                    nc.vector.tensor_scalar(out=biasT_s[:, mc, :], in0=tt,
                                            scalar1=bias_scale,
                                            scalar2=mb_s[:, mc:mc + 1],
                                            op0=ALU.mult, op1=ALU.add)
                else:
                    nc.vector.tensor_scalar_mul(out=biasT_s[:, mc, :], in0=tt,
                                                scalar1=bias_scale)

        # ================= Phase C: QKV projections =================
        nc.vector.memset(vaug_s[:, :, :, DH:DH + 1], 1.0)
        with tc.tile_pool(name="ph_cq_ps", bufs=4, space="PSUM") as pcps:
            # Q^T (own 512 rows), scaled by 1/8
            for jc in range(KO):
                qp = pcps.tile([128, SQ], F32, tag="q")
                for ko in range(KO):
                    nc.tensor.matmul(qp, wq_s[:, ko, jc * 128:(jc + 1) * 128],
                                     hT_s[:, ko, :SQ],
                                     start=(ko == 0), stop=(ko == KO - 1))
                nc.scalar.activation(out=QT_s[:, jc, :], in_=qp, func=AF.Identity,
                                     bias=bq_s[:, jc:jc + 1], scale=0.125)
        with tc.tile_pool(name="ph_ck_ps", bufs=4, space="PSUM") as pcps2:
            # K^T (all 1024 rows)
            for jc in range(KO):
                for n2 in range(2):
                    kp = pcps2.tile([128, 512], F32, tag="k")
                    for ko in range(KO):
                        nc.tensor.matmul(kp, wk_s[:, ko, jc * 128:(jc + 1) * 128],
                                         hT_s[:, ko, n2 * 512:(n2 + 1) * 512],
                                         start=(ko == 0), stop=(ko == KO - 1))
                    nc.scalar.activation(out=KT_s[:, jc, n2 * 512:(n2 + 1) * 512],
                                         in_=kp, func=AF.Identity,
                                         bias=bk_s[:, jc:jc + 1])
        with tc.tile_pool(name="ph_cv_ps", bufs=4, space="PSUM") as pcps3:
            # V natural layout (all 1024 rows), written per-head with ones col
            for sc in range(MC):
                for j0, jn in ((0, 512), (512, 512), (1024, 256)):
                    vp = pcps3.tile([128, 512], F32, tag="v")
                    for ko in range(KO):
                        nc.tensor.matmul(vp[:, :jn],
                                         hT_s[:, ko, sc * 128:(sc + 1) * 128],
                                         wv_s[:, ko, j0:j0 + jn],
                                         start=(ko == 0), stop=(ko == KO - 1))
                    nc.scalar.activation(
                        out=vaug_s[:, sc, j0 // DH:(j0 + jn) // DH, 0:DH],
                        in_=vp[:, :jn].rearrange("p (h d) -> p h d", d=DH),
                        func=AF.Copy)
        es_hT.close()  # free h^T
        es_w.close()   # free QKV weights

        # ================= Phase D: attention =================
        es_ctx = ExitStack()      # phases D..E
        p_ctx = es_ctx.enter_context(tc.tile_pool(name="p_ctx", bufs=1, side="right"))
        ctxn_s = p_ctx.tile([128, LC, H], BF)          # attention out, natural
        with tc.tile_pool(name="ph_d", bufs=3) as pd, \
             tc.tile_pool(name="ph_d_pr", bufs=2) as pdp, \
             tc.tile_pool(name="ph_d_ps", bufs=2, space="PSUM") as pdps, \
             tc.tile_pool(name="ph_d_pst", bufs=2, space="PSUM") as pdpst, \
             tc.tile_pool(name="ph_d_ps2", bufs=2, space="PSUM") as pdps2:
            for hp in range(NH // 2):
                ko = hp
                pts = [pdp.tile([128, MC, SQ], BF, tag=f"probsT{i}",
                                name=f"pt_{hp}_{i}") for i in range(2)]
                for mc in range(MC):
                    sps = [pdps.tile([128, SQ], F32, tag=f"sc{i}",
                                     name=f"sp_{hp}_{mc}_{i}") for i in range(2)]
                    # adjacent K=64 matmuls at base partitions 0 / 64 run
                    # concurrently in distinct PE row-groups
                    for i in range(2):
                        p0 = i * DH
                        nc.tensor.matmul(sps[i],
                                         KT_s[p0:p0 + DH, ko,
                                              mc * 128:(mc + 1) * 128],
                                         QT_s[p0:p0 + DH, ko, :],
                                         start=True, stop=False)
                    for i in range(2):
                        nc.tensor.matmul(sps[i], ident_b, biasT_s[:, mc, :],
                                         start=False, stop=True)
                        nc.scalar.activation(out=pts[i][:, mc, :], in_=sps[i],
                                             func=AF.Exp)
                for i in range(2):
                    hh = 2 * hp + i
                    cp = pdps2.tile([DH + 1, SQ], F32, tag="cx",
                                    name=f"cp_{hh}")
                    for mc in range(MC):
                        nc.tensor.matmul(cp, vaug_s[:, mc, hh, :],
                                         pts[i][:, mc, :],
                                         start=(mc == 0), stop=(mc == MC - 1))
                    cs = pd.tile([DH + 1, SQ], BF, tag="cs", name=f"cs_{hh}")
                    nc.vector.tensor_copy(out=cs, in_=cp)
                    for lc in range(LC):
                        tp = pdpst.tile([128, DH + 1], BF, tag="ct",
                                        name=f"ct_{hh}_{lc}")
                        nc.tensor.transpose(tp, cs[:, lc * 128:(lc + 1) * 128],
                                            ident_b[:DH + 1, :DH + 1])
                        rc = pd.tile([128, 1], F32, tag="rc",
                                     name=f"rc_{hh}_{lc}")
                        nc.vector.reciprocal(out=rc, in_=tp[:, DH:DH + 1])
                        nc.vector.tensor_scalar_mul(
                            out=ctxn_s[:, lc, hh * DH:(hh + 1) * DH],
                            in0=tp[:, 0:DH], scalar1=rc)

        es_attn.close()  # free biasT/QT/KT/V

        # ================= Phase E: residual + LN =================
        es_x = ExitStack()        # phases E..G
        p_x = es_x.enter_context(tc.tile_pool(name="p_x", bufs=1))
        x2_s = p_x.tile([128, LC, H], F32)             # LN out (+beta+bf2)
        xT_s = p_x.tile([128, KO, SQ], BF)             # x2^T
        gT_s = p_x.tile([128, IC, SQ], BF)             # gelu(ffn1)^T
        with tc.tile_pool(name="ph_e", bufs=2) as pe, \
             tc.tile_pool(name="ph_e_ps", bufs=4, space="PSUM") as peps:
            for lc in range(LC):
                hr = pe.tile([128, H], F32, tag="hr")
                nc.sync.dma_start(out=hr,
                                  in_=hres_d.ap()[lc * 128:(lc + 1) * 128, :])
                xs = x2_s[:, lc, :]
                nc.vector.tensor_add(out=xs, in0=hr, in1=ctxn_s[:, lc, :])
                st = pe.tile([128, 5, 6], F32, tag="st")
                xg = xs.rearrange("p (g d) -> p g d", d=256)
                for sg in range(5):
                    nc.vector.bn_stats(out=st[:, sg, :], in_=xg[:, sg, :])
                mv = pe.tile([128, 2], F32, tag="mv")
                nc.vector.bn_aggr(out=mv, in_=st)
                sd = pe.tile([128, 1], F32, tag="sd")
                nc.scalar.activation(out=sd, in_=mv[:, 1:2], func=AF.Sqrt,
                                     bias=eps_s)
                rs = pe.tile([128, 1], F32, tag="rs")
                nc.vector.reciprocal(out=rs, in_=sd)
                nc.vector.tensor_scalar(out=xs, in0=xs, scalar1=mv[:, 0:1],
                                        scalar2=rs, op0=ALU.subtract, op1=ALU.mult)
                nc.gpsimd.tensor_mul(out=xs, in0=xs, in1=lng_b)
                nc.gpsimd.tensor_add(out=xs, in0=xs, in1=lnbf_b)
                for ko in range(KO):
                    tp = peps.tile([128, 128], F32, tag="xt")
                    nc.tensor.transpose(tp, xs[:, ko * 128:(ko + 1) * 128], ident_f)
                    nc.vector.tensor_copy(
                        out=xT_s[:, ko, lc * 128:(lc + 1) * 128], in_=tp)
        es_ctx.close()  # free ctxn

        # ================= Phase F: FFN1 (gelu) =================
        with tc.tile_pool(name="ph_f_w", bufs=8) as pfw, \
             tc.tile_pool(name="ph_f_ps", bufs=4, space="PSUM") as pfps:
            for ic2 in range(IC // 2):
                wt = pfw.tile([128, KO, 256], BF, tag="w1")
                eng = nc.sync if ic2 % 2 == 0 else nc.gpsimd
                eng.dma_start(out=wt, in_=wf1T_d.ap()[ic2])
                for i_in in range(2):
                    ic = ic2 * 2 + i_in
                    yp = pfps.tile([128, SQ], F32, tag="y")
                    for ko in range(KO):
                        nc.tensor.matmul(yp,
                                         wt[:, ko, i_in * 128:(i_in + 1) * 128],
                                         xT_s[:, ko, :],
                                         start=(ko == 0), stop=(ko == KO - 1))
                    nc.scalar.activation(out=gT_s[:, ic, :], in_=yp, func=AF.Gelu,
                                         bias=bf1_s[:, ic:ic + 1])

        # ================= Phase G: FFN2 + residual + store =================
        with tc.tile_pool(name="ph_g_w", bufs=8) as pgw, \
             tc.tile_pool(name="ph_g_o", bufs=3) as pgo, \
             tc.tile_pool(name="ph_g_ps", bufs=1, space="PSUM") as pgps:
            for j0, jn in ((0, 512), (512, 512), (1024, 256)):
                zps = [pgps.tile([128, jn], F32, tag=f"z{lc}", name=f"zp_{j0}_{lc}")
                       for lc in range(LC)]
                for ic in range(IC):
                    w2 = pgw.tile([128, 512], BF, tag="w2")
                    eng = nc.sync if ic % 2 == 0 else nc.gpsimd
                    eng.dma_start(out=w2[:, :jn],
                                  in_=wf2T_d.ap()[ic, :, j0:j0 + jn])
                    for lc in range(LC):
                        nc.tensor.matmul(zps[lc],
                                         gT_s[:, ic, lc * 128:(lc + 1) * 128],
                                         w2[:, :jn],
                                         start=(ic == 0), stop=(ic == IC - 1))
                for lc in range(LC):
                    ot = pgo.tile([128, 512], F32, tag="ot")
                    nc.vector.tensor_add(out=ot[:, :jn], in0=zps[lc],
                                         in1=x2_s[:, lc, j0:j0 + jn])
                    nc.sync.dma_start(
                        out=out_d.ap()[lc * 128:(lc + 1) * 128, j0:j0 + jn],
                        in_=ot[:, :jn])
        es_x.close()

    nc.compile()
    return nc


_NC_CACHE = {}


def _get_nc(use_mask: bool, bias_scale: float):
    key = (use_mask, round(bias_scale, 9))
    if key not in _NC_CACHE:
        _NC_CACHE[key] = build_nc(use_mask, bias_scale)
    return _NC_CACHE[key]


def _prep_inputs(inputs):
    f32 = lambda x: np.ascontiguousarray(np.asarray(x, np.float32))
    bft = lambda x: np.ascontiguousarray(np.asarray(x, np.float32).T).astype(BF16)
    hs = f32(inputs["hidden_states"])
    mask = f32(inputs["attention_mask"])
    M, W1, b1, W2, b2 = (f32(inputs["M"]), f32(inputs["W_ct1"]),
                         f32(inputs["b_ct1"]), f32(inputs["W_ct2"]),
                         f32(inputs["b_ct2"]))
    R = ((M.T @ W1.T + b1).T @ (M @ W2.T + b2)).astype(np.float32)
    bias_scale = float(np.asarray(inputs["bias_scale"]).reshape(-1)[0])
    use_mask = not bool(np.all(mask == 1.0))

    def pack_kxj(wT):
        # (H, J) -> (128, KO, J): partition-major, contiguous per partition
        return np.ascontiguousarray(
            wT.reshape(KO, 128, wT.shape[1]).transpose(1, 0, 2))

    wf1T = bft(inputs["Wf1"])                     # (H, I)
    wf1p = np.ascontiguousarray(
        wf1T.reshape(KO, 128, IC // 2, 256).transpose(2, 0, 1, 3)
            .transpose(0, 2, 1, 3))               # (IC//2, 128, KO, 256)
    wf2T = bft(inputs["Wf2"])                     # (I, H)
    wf2p = np.ascontiguousarray(wf2T.reshape(IC, 128, H))
    shared = {
        "wqT": pack_kxj(bft(inputs["Wq"])), "wkT": pack_kxj(bft(inputs["Wk"])),
        "wvT": pack_kxj(bft(inputs["Wv"])), "wptmT": bft(inputs["W_ptm"]),
        "rmat": np.ascontiguousarray(R).astype(BF16),
        "wf1T": wf1p, "wf2T": wf2p,
        "bq": f32(inputs["bq"]).reshape(KO, 128),
        "bk": f32(inputs["bk"]).reshape(KO, 128),
        "bptm": f32(inputs["b_ptm"]).reshape(P, 1),
        "bf1": f32(inputs["bf1"]).reshape(IC, 128),
        "lng": f32(inputs["ln_g"]),
        "lnbf": f32(inputs["ln_b"]) + f32(inputs["bf2"]),
    }
    bv = f32(inputs["bv"])
    in_maps = []
    for c in range(8):
        b, half = c // 2, c % 2
        r0 = half * SQ
        mb = np.roll((1.0 - mask[b]) * np.float32(-1e30), -r0)
        m = dict(shared)
        m["h"] = np.ascontiguousarray(np.roll(hs[b], -r0, axis=0))
        m["hres"] = np.ascontiguousarray(hs[b, r0:r0 + SQ] + bv[None, :])
        m["mb"] = np.ascontiguousarray(mb.reshape(MC, 128))
        in_maps.append(m)
    return in_maps, use_mask, bias_scale


def kernel(**inputs) -> np.ndarray:
    in_maps, use_mask, bias_scale = _prep_inputs(inputs)
    nc = _get_nc(use_mask, bias_scale)
    res = run_bass_kernel_spmd(nc, in_maps, list(range(8)))
    out = np.zeros((B, S, H), np.float32)
    for c in range(8):
        b, half = c // 2, c % 2
        r0 = half * SQ
        out[b, r0:r0 + SQ] = res.results[c]["out"]
    return out



# revision 11
# speedup vs baseline: 1.3424x; 1.3424x over previous
"""Trainium2 Bass kernel for the custom transformer layer.

Sharding: 8 cores = 4 batches x 2 query-row halves. Each core computes the
full layer for 512 query rows of one batch. K/V/ptm are computed for the
whole batch on both cores sharing it (duplicated; no cross-core traffic).
Inputs are rotated per-core so own query rows sit at positions 0..511 ->
one SPMD program for all cores.

v2: fp8 (e4m3) DoubleRow matmuls for the QKV projections, ptm logits and
the probs@V contraction (2x PE throughput); h^T is pre-transposed on the
host and DMA'd directly (kills the fp32 transpose phase); the attention
bias is applied as exp(bias) multiplied into exp(scores) on the DVE
instead of an identity-matmul accumulation on the PE; softmax of ptm is
normalized via a PE broadcast matmul instead of a DRAM roundtrip; the LN
gamma/beta are folded into Wf1/bf1 so the FFN path needs only the
un-affine LN output; FFN stays bf16 for precision. Phase B (bias) is
emission-interleaved with phase C (QKV) to keep the PE fed.
"""

import sys

sys.path.insert(0, "/opt/trn_rl_repo")

import numpy as np
import ml_dtypes

import concourse.bass as bass
import concourse.tile as tile
from concourse import bacc, mybir
from concourse.bass_utils import run_bass_kernel_spmd
from concourse.masks import make_identity

BF16 = ml_dtypes.bfloat16
F8NP = ml_dtypes.float8_e4m3
F32 = mybir.dt.float32
BF = mybir.dt.bfloat16
F8 = mybir.dt.float8e4
AF = mybir.ActivationFunctionType
ALU = mybir.AluOpType
DR = mybir.MatmulPerfMode.DoubleRow

B, S, H, NH, DH, I, C, P = 4, 1024, 1280, 20, 64, 5120, 13, 8
SQ = 512          # query rows per core
KO = H // 128     # 10 k-chunks
IC = I // 128     # 40 i-chunks
LC = SQ // 128    # 4 l-chunks
MC = S // 128     # 8 m-chunks
LN_EPS = 1e-5
WS = 32.0         # fp8 weight prescale
G2 = ((0, 512), (512, 512), (1024, 256))   # ffn2 output column groups


def bcast_ap(src: bass.AP, parts: int) -> bass.AP:
    """Partition-stride-0 broadcast AP (for DMA sources)."""
    return bass.AP(tensor=src.tensor, offset=src.offset,
                   ap=[[0, parts]] + [list(d) for d in src.ap])


def build_nc(bias_scale: float):
    nc = bacc.Bacc("TRN2", target_bir_lowering=False, debug=False, num_devices=8)

    # ---- DRAM parameters (per-core) ----
    hT_d = nc.declare_dram_parameter("hT", [128, KO, S], F8, isOutput=False)
    hres_d = nc.declare_dram_parameter("hres", [SQ, H], F32, isOutput=False)
    wqT_d = nc.declare_dram_parameter("wqT", [128, KO // 2, KO, 2, 128], F8,
                                      isOutput=False)
    wkT_d = nc.declare_dram_parameter("wkT", [128, KO // 2, KO, 2, 128], F8,
                                      isOutput=False)
    wvT_d = nc.declare_dram_parameter("wvT", [128, KO, H], F8, isOutput=False)
    hTp_d = nc.declare_dram_parameter("hTp", [128, KO // 2, MC, 2, 128], F8,
                                      isOutput=False)
    wptmT_d = nc.declare_dram_parameter("wptmT", [128, KO // 2, 2, 64], F8,
                                      isOutput=False)
    rmat_d = nc.declare_dram_parameter("rmat", [P, P], BF, isOutput=False)
    wf1T_d = nc.declare_dram_parameter("wf1T", [IC // 2, 128, KO, 256], BF,
                                       isOutput=False)
    wf2T_d = nc.declare_dram_parameter("wf2T", [IC, 128, H], BF, isOutput=False)
    bq_d = nc.declare_dram_parameter("bq", [KO, 128], F32, isOutput=False)
    bk_d = nc.declare_dram_parameter("bk", [KO, 128], F32, isOutput=False)
    bptm_d = nc.declare_dram_parameter("bptm", [P, 1], F32, isOutput=False)
    bf1_d = nc.declare_dram_parameter("bf1", [IC, 128], F32, isOutput=False)
    lng_d = nc.declare_dram_parameter("lng", [H], F32, isOutput=False)
    lnbf_d = nc.declare_dram_parameter("lnbf", [H], F32, isOutput=False)
    mb_d = nc.declare_dram_parameter("mb", [MC, 128], F32, isOutput=False)
    out_d = nc.declare_dram_parameter("out", [SQ, H], F32, isOutput=True)

    from contextlib import ExitStack
    with tile.TileContext(nc) as tc, ExitStack() as es:
        # ---- persistent constants ----
        const = es.enter_context(tc.tile_pool(name="const", bufs=1))
        ident_b = const.tile([128, 128], BF)
        make_identity(nc, ident_b)
        wptm_s = const.tile([128, KO // 2, 2, 64], F8)
        nc.sync.dma_start(out=wptm_s, in_=wptmT_d.ap())
        bq_s = const.tile([128, KO], F32)
        nc.sync.dma_start(out=bq_s, in_=bq_d.ap().rearrange("c p -> p c"))
        bk_s = const.tile([128, KO], F32)
        nc.sync.dma_start(out=bk_s, in_=bk_d.ap().rearrange("c p -> p c"))
        bf1_s = const.tile([128, IC], F32)
        nc.sync.dma_start(out=bf1_s, in_=bf1_d.ap().rearrange("c p -> p c"))
        bptm_s = const.tile([P, 1], F32)
        nc.sync.dma_start(out=bptm_s, in_=bptm_d.ap())
        mb_s = const.tile([128, MC], F32)
        nc.sync.dma_start(out=mb_s, in_=mb_d.ap().rearrange("c p -> p c"))
        rmat_s = const.tile([P, P], BF)
        nc.sync.dma_start(out=rmat_s, in_=rmat_d.ap())
        lng_b = const.tile([128, H], F32)
        nc.sync.dma_start(out=lng_b, in_=bcast_ap(lng_d.ap(), 128))
        lnbf_b = const.tile([128, H], F32)
        nc.sync.dma_start(out=lnbf_b, in_=bcast_ap(lnbf_d.ap(), 128))
        eps_s = const.tile([128, 1], F32)
        nc.vector.memset(eps_s, LN_EPS)
        ones8_s = const.tile([P, 1], BF)
        nc.vector.memset(ones8_s, 1.0)
        ones1_s = const.tile([1, P], F32)
        nc.vector.memset(ones1_s, 1.0)

        # ---- big prefetches, 4 parallel DMA queues ----
        es_hT = ExitStack()
        p_hT = es_hT.enter_context(tc.tile_pool(name="p_hT", bufs=1, side="right"))
        hT_s = p_hT.tile([128, KO, S], F8)             # h^T, feature-major
        for n2 in range(2):
            nc.sync.dma_start(out=hT_s[:, :, n2 * 512:(n2 + 1) * 512],
                              in_=hT_d.ap()[:, :, n2 * 512:(n2 + 1) * 512])
        es_w = ExitStack()
        p_w = es_w.enter_context(tc.tile_pool(name="p_w", bufs=1, side="right"))
        wq_s = p_w.tile([128, KO // 2, KO, 2, 128], F8)
        wk_s = p_w.tile([128, KO // 2, KO, 2, 128], F8)
        wv_s = p_w.tile([128, KO, H], F8)
        hTp_s = p_w.tile([128, KO // 2, MC, 2, 128], F8)
        nc.scalar.dma_start(out=wq_s, in_=wqT_d.ap())
        nc.sync.dma_start(out=wk_s, in_=wkT_d.ap())
        nc.gpsimd.dma_start(out=wv_s, in_=wvT_d.ap())
        nc.gpsimd.dma_start(out=hTp_s, in_=hTp_d.ap())

        # ---- persistent activations (attention scope) ----
        es_attn = ExitStack()
        p_attn = es_attn.enter_context(tc.tile_pool(name="p_attn", bufs=1))
        expbT_s = p_attn.tile([128, MC, SQ], BF)       # exp(attn bias), [m, l]
        QT_s = p_attn.tile([128, KO, SQ], F8)          # (q/8)^T
        KT_s = p_attn.tile([128, KO, S], F8)           # k^T
        # V with a ones column per head: [s-part, s-chunk, head, 64+1]
        vaug_s = p_attn.tile([128, MC, NH, DH + 1], BF)

        # ======== Phases B (bias) + C (QKV), emission-interleaved ========
        with tc.tile_pool(name="ph_b", bufs=1) as pb, \
             tc.tile_pool(name="ph_b_ps", bufs=2, space="PSUM") as pbps, \
             tc.tile_pool(name="ph_c_ps", bufs=4, space="PSUM") as pcps:
            # B1: ptm logits -> expT (bf16)
            expT_s = pb.tile([P, S], BF, tag="expT")
            for n2 in range(2):
                lp = pbps.tile([64, 512], F32, tag="b", name=f"lp_{n2}")
                for k2 in range(KO // 2):
                    nc.tensor.matmul(lp, wptm_s[:, k2, :, :],
                                     hT_s[:, 2 * k2:2 * k2 + 2,
                                          n2 * 512:(n2 + 1) * 512],
                                     start=(k2 == 0), stop=(k2 == KO // 2 - 1),
                                     perf_mode=DR)
                nc.scalar.activation(out=expT_s[:, n2 * 512:(n2 + 1) * 512],
                                     in_=lp[0:P, :], func=AF.Exp, bias=bptm_s,
                                     scale=1.0 / WS)
            # C-Q: Q^T (own 512 rows), scaled by 1/8
            for jc in range(KO):
                qp = pcps.tile([128, SQ], F32, tag="c", name=f"qp_{jc}")
                for k2 in range(KO // 2):
                    nc.tensor.matmul(qp, wq_s[:, k2, jc, :, :],
                                     hT_s[:, 2 * k2:2 * k2 + 2, :SQ],
                                     start=(k2 == 0), stop=(k2 == KO // 2 - 1),
                                     perf_mode=DR)
                nc.scalar.activation(out=QT_s[:, jc, :], in_=qp, func=AF.Identity,
                                     bias=bq_s[:, jc:jc + 1], scale=0.125 / WS)
            # B2: ptm softmax normalizer; broadcast 1/Z via PE ones-matmul
            rz_s = pb.tile([1, S], F32, tag="rz")
            for n2 in range(2):
                zp = pbps.tile([1, 512], F32, tag="b", name=f"zp_{n2}")
                nc.tensor.matmul(zp, ones8_s,
                                 expT_s[:, n2 * 512:(n2 + 1) * 512],
                                 start=True, stop=True)
                nc.vector.reciprocal(out=rz_s[:, n2 * 512:(n2 + 1) * 512], in_=zp)
            ptmT_s = pb.tile([P, S], BF, tag="ptmT")
            for n2 in range(2):
                zbp = pbps.tile([P, 512], F32, tag="b", name=f"zbp_{n2}")
                nc.tensor.matmul(zbp, ones1_s,
                                 rz_s[:, n2 * 512:(n2 + 1) * 512],
                                 start=True, stop=True)
                nc.vector.tensor_mul(out=ptmT_s[:, n2 * 512:(n2 + 1) * 512],
                                     in0=expT_s[:, n2 * 512:(n2 + 1) * 512],
                                     in1=zbp)
            # C-K first half
            for jc in range(KO // 2):
                for n2 in range(2):
                    kp = pcps.tile([128, 512], F32, tag="c",
                                   name=f"kp_{jc}_{n2}")
                    for k2 in range(KO // 2):
                        nc.tensor.matmul(
                            kp, wk_s[:, k2, jc, :, :],
                            hT_s[:, 2 * k2:2 * k2 + 2, n2 * 512:(n2 + 1) * 512],
                            start=(k2 == 0), stop=(k2 == KO // 2 - 1),
                            perf_mode=DR)
                    nc.scalar.activation(out=KT_s[:, jc, n2 * 512:(n2 + 1) * 512],
                                         in_=kp, func=AF.Identity,
                                         bias=bk_s[:, jc:jc + 1], scale=1.0 / WS)
            # B3: g = R @ ptm^T (own rows)
            gTs = pb.tile([P, SQ], BF, tag="gTs")
            gp = pbps.tile([P, 512], F32, tag="b", name="gp")
            nc.tensor.matmul(gp, rmat_s, ptmT_s[:, :SQ], start=True, stop=True)
            nc.vector.tensor_copy(out=gTs, in_=gp)
            # C-K second half
            for jc in range(KO // 2, KO):
                for n2 in range(2):
                    kp = pcps.tile([128, 512], F32, tag="c",
                                   name=f"kp_{jc}_{n2}")
                    for k2 in range(KO // 2):
                        nc.tensor.matmul(
                            kp, wk_s[:, k2, jc, :, :],
                            hT_s[:, 2 * k2:2 * k2 + 2, n2 * 512:(n2 + 1) * 512],
                            start=(k2 == 0), stop=(k2 == KO // 2 - 1),
                            perf_mode=DR)
                    nc.scalar.activation(out=KT_s[:, jc, n2 * 512:(n2 + 1) * 512],
                                         in_=kp, func=AF.Identity,
                                         bias=bk_s[:, jc:jc + 1], scale=1.0 / WS)
            # B4: u = ptm^T_mc . g, tanh (batched), then exp (batched)
            tt_s = pb.tile([128, MC, SQ], BF, tag="tt")
            for mc in range(MC):
                up = pbps.tile([128, SQ], F32, tag="b", name=f"up_{mc}")
                nc.tensor.matmul(up, ptmT_s[:, mc * 128:(mc + 1) * 128], gTs,
                                 start=True, stop=True)
                nc.scalar.activation(out=tt_s[:, mc, :], in_=up, func=AF.Tanh)
            for mc in range(MC):
                nc.scalar.activation(out=expbT_s[:, mc, :], in_=tt_s[:, mc, :],
                                     func=AF.Exp, scale=bias_scale,
                                     bias=mb_s[:, mc:mc + 1])
            # C-V: natural layout (all 1024 rows), per-head with ones col
            nc.vector.memset(vaug_s[:, :, :, DH:DH + 1], 1.0)
            for sc in range(MC):
                for j0, jn in G2:
                    vp = pcps.tile([128, 512], F32, tag="c",
                                   name=f"vp_{sc}_{j0}")
                    for k2 in range(KO // 2):
                        nc.tensor.matmul(vp[:, :jn],
                                         hTp_s[:, k2, sc, :, :],
                                         wv_s[:, 2 * k2:2 * k2 + 2, j0:j0 + jn],
                                         start=(k2 == 0), stop=(k2 == KO // 2 - 1),
                                         perf_mode=DR)
                    nc.scalar.activation(
                        out=vaug_s[:, sc, j0 // DH:(j0 + jn) // DH, 0:DH],
                        in_=vp[:, :jn].rearrange("p (h d) -> p h d", d=DH),
                        func=AF.Copy, scale=1.0 / WS)
        es_w.close()   # free QKV weights
        es_hT.close()  # free h^T

        # ================= Phase D: attention =================
        es_ctx = ExitStack()      # phases D..E
        p_ctx = es_ctx.enter_context(tc.tile_pool(name="p_ctx", bufs=1,
                                                  side="right"))
        ctxn_s = p_ctx.tile([128, LC, H], BF)          # attention out, natural
        hres_s = p_ctx.tile([128, LC, H], F32)         # h residual (+bv)
        nc.sync.dma_start(out=hres_s,
                          in_=hres_d.ap().rearrange("(lc p) h -> p lc h", p=128))
        with tc.tile_pool(name="ph_d", bufs=4) as pd, \
             tc.tile_pool(name="ph_d_pr", bufs=2) as pdp, \
             tc.tile_pool(name="ph_d_ps", bufs=4, space="PSUM") as pdps, \
             tc.tile_pool(name="ph_d_pst", bufs=2, space="PSUM") as pdpst, \
             tc.tile_pool(name="ph_d_ps2", bufs=2, space="PSUM") as pdps2:
            for hp in range(NH // 2):
                ko = hp
                pts = [pdp.tile([128, MC, SQ], BF, tag=f"probsT{i}",
                                name=f"pt_{hp}_{i}") for i in range(2)]
                for mc in range(MC):
                    for i in range(2):
                        p0 = i * DH
                        # adjacent K=64 matmuls at base partitions 0 / 64 run
                        # concurrently in distinct PE row-groups
                        sp = pdps.tile([128, SQ], F32, tag="sc",
                                       name=f"sp_{hp}_{mc}_{i}")
                        nc.tensor.matmul(sp,
                                         KT_s[p0:p0 + DH, ko,
                                              mc * 128:(mc + 1) * 128],
                                         QT_s[p0:p0 + DH, ko, :],
                                         start=True, stop=True)
                        et = pd.tile([128, SQ], BF, tag="es",
                                     name=f"es_{hp}_{mc}_{i}")
                        nc.scalar.activation(out=et, in_=sp, func=AF.Exp)
                        nc.vector.tensor_mul(out=pts[i][:, mc, :], in0=et,
                                             in1=expbT_s[:, mc, :])
                for i in range(2):
                    hh = 2 * hp + i
                    cp = pdps2.tile([DH + 1, SQ], F32, tag="cx",
                                    name=f"cp_{hh}")
                    for mc in range(MC):
                        nc.tensor.matmul(cp, vaug_s[:, mc, hh, :],
                                         pts[i][:, mc, :],
                                         start=(mc == 0), stop=(mc == MC - 1))
                    cs = pd.tile([DH + 1, SQ], BF, tag="cs", name=f"cs_{hh}")
                    nc.vector.tensor_copy(out=cs, in_=cp)
                    for lc in range(LC):
                        tp = pdpst.tile([128, DH + 1], BF, tag="ct",
                                        name=f"ct_{hh}_{lc}")
                        nc.tensor.transpose(tp, cs[:, lc * 128:(lc + 1) * 128],
                                            ident_b[:DH + 1, :DH + 1])
                        rc = pd.tile([128, 1], F32, tag="rc",
                                     name=f"rc_{hh}_{lc}")
                        nc.vector.reciprocal(out=rc, in_=tp[:, DH:DH + 1])
                        nc.vector.tensor_scalar_mul(
                            out=ctxn_s[:, lc, hh * DH:(hh + 1) * DH],
                            in0=tp[:, 0:DH], scalar1=rc)
        es_attn.close()  # free expbT/QT/KT/V

        # ================= Phase E: residual + LN =================
        es_x = ExitStack()        # phases E..G
        p_x = es_x.enter_context(tc.tile_pool(name="p_x", bufs=1))
        x2_s = p_x.tile([128, LC, H], F32)             # x, then LN-affine out
        xn_s = p_x.tile([128, LC, H], BF)              # un-affine LN out
        xT_s = p_x.tile([128, KO, SQ], BF)             # xn^T
        gT_s = p_x.tile([128, IC, SQ], BF)             # gelu(ffn1)^T
        with tc.tile_pool(name="ph_e", bufs=4) as pe, \
             tc.tile_pool(name="ph_e_ps", bufs=4, space="PSUM") as peps:
            mvs, rss = [], []
            for lc in range(LC):
                xs = x2_s[:, lc, :]
                nc.gpsimd.tensor_add(out=xs, in0=hres_s[:, lc, :],
                                     in1=ctxn_s[:, lc, :])
                st = pe.tile([128, 5, 6], F32, tag="st", name=f"st_{lc}")
                xg = xs.rearrange("p (g d) -> p g d", d=256)
                for sg in range(5):
                    nc.vector.bn_stats(out=st[:, sg, :], in_=xg[:, sg, :])
                mv = pe.tile([128, 2], F32, tag="mv", name=f"mv_{lc}")
                nc.vector.bn_aggr(out=mv, in_=st)
                mvs.append(mv)
            for lc in range(LC):
                sd = pe.tile([128, 1], F32, tag="sd", name=f"sd_{lc}")
                nc.scalar.activation(out=sd, in_=mvs[lc][:, 1:2], func=AF.Sqrt,
                                     bias=eps_s)
                rss.append(sd)
            for lc in range(LC):
                rs = pe.tile([128, 1], F32, tag="rs", name=f"rs_{lc}")
                nc.vector.reciprocal(out=rs, in_=rss[lc])
                nc.vector.tensor_scalar(out=xn_s[:, lc, :], in0=x2_s[:, lc, :],
                                        scalar1=mvs[lc][:, 0:1], scalar2=rs,
                                        op0=ALU.subtract, op1=ALU.mult)
                for ko in range(KO):
                    tp = peps.tile([128, 128], BF, tag="xt",
                                   name=f"xt_{lc}_{ko}")
                    nc.tensor.transpose(
                        tp, xn_s[:, lc, ko * 128:(ko + 1) * 128], ident_b)
                    nc.scalar.activation(
                        out=xT_s[:, ko, lc * 128:(lc + 1) * 128], in_=tp,
                        func=AF.Copy)
            for lc in range(LC):
                # LN affine for the residual path only (g/b folded into FFN1)
                nc.gpsimd.tensor_mul(out=x2_s[:, lc, :], in0=xn_s[:, lc, :],
                                     in1=lng_b)
                nc.gpsimd.tensor_add(out=x2_s[:, lc, :], in0=x2_s[:, lc, :],
                                     in1=lnbf_b)
        es_ctx.close()  # free ctxn/hres

        # ====== Phase F: FFN1 (gelu) + FFN2 weight prefetch ======
        es_g = ExitStack()
        p_gw = es_g.enter_context(tc.tile_pool(name="p_gw", bufs=1))
        w2_tiles = {}
        w2_engs = (nc.sync, nc.gpsimd, nc.scalar)

        def w2_load(gidx, eng):
            gi2, icw = divmod(gidx, IC)
            j0w, jnw = G2[gi2]
            t = p_gw.tile([128, 512], BF, tag="w2", bufs=16,
                          name=f"w2_{gidx}")
            eng.dma_start(out=t[:, :jnw], in_=wf2T_d.ap()[icw, :, j0w:j0w + jnw])
            w2_tiles[gidx] = t

        with tc.tile_pool(name="ph_f_w", bufs=8) as pfw, \
             tc.tile_pool(name="ph_f_ps", bufs=4, space="PSUM") as pfps:
            for ic2 in range(IC // 2):
                wt = pfw.tile([128, KO, 256], BF, tag="w1", name=f"w1_{ic2}")
                eng = nc.sync if ic2 % 2 == 0 else nc.gpsimd
                eng.dma_start(out=wt, in_=wf1T_d.ap()[ic2])
                if ic2 >= 12:
                    w2_load(2 * (ic2 - 12), nc.scalar)
                    w2_load(2 * (ic2 - 12) + 1, nc.scalar)
                for i_in in range(2):
                    ic = ic2 * 2 + i_in
                    yp = pfps.tile([128, SQ], F32, tag="y", name=f"yp_{ic}")
                    for ko in range(KO):
                        nc.tensor.matmul(yp,
                                         wt[:, ko, i_in * 128:(i_in + 1) * 128],
                                         xT_s[:, ko, :],
                                         start=(ko == 0), stop=(ko == KO - 1))
                    nc.scalar.activation(out=gT_s[:, ic, :], in_=yp, func=AF.Gelu,
                                         bias=bf1_s[:, ic:ic + 1])

        # ================= Phase G: FFN2 + residual + store =================
        NW2 = len(G2) * IC
        with tc.tile_pool(name="ph_g_o", bufs=3) as pgo, \
             tc.tile_pool(name="ph_g_ps", bufs=2, space="PSUM") as pgps:
            for gi, (j0, jn) in enumerate(G2):
                zps = [pgps.tile([128, jn], F32, tag=f"z{lc}",
                                 name=f"zp_{j0}_{lc}") for lc in range(LC)]
                for ic in range(IC):
                    gidx = gi * IC + ic
                    pf = gidx + 16
                    if pf < NW2:
                        w2_load(pf, w2_engs[gidx % 3])
                    w2 = w2_tiles.pop(gidx)
                    for lc in range(LC):
                        nc.tensor.matmul(zps[lc],
                                         gT_s[:, ic, lc * 128:(lc + 1) * 128],
                                         w2[:, :jn],
                                         start=(ic == 0), stop=(ic == IC - 1))
                for lc in range(LC):
                    ot = pgo.tile([128, 512], F32, tag="ot",
                                  name=f"ot_{j0}_{lc}")
                    nc.vector.tensor_add(out=ot[:, :jn], in0=zps[lc],
                                         in1=x2_s[:, lc, j0:j0 + jn])
                    eng = w2_engs[lc % 3]
                    eng.dma_start(
                        out=out_d.ap()[lc * 128:(lc + 1) * 128, j0:j0 + jn],
                        in_=ot[:, :jn])
        es_g.close()
        es_x.close()

    nc.compile()
    return nc


_NC_CACHE = {}


def _get_nc(bias_scale: float):
    key = round(bias_scale, 9)
    if key not in _NC_CACHE:
        _NC_CACHE[key] = build_nc(bias_scale)
    return _NC_CACHE[key]


def _prep_inputs(inputs):
    f32 = lambda x: np.ascontiguousarray(np.asarray(x, np.float32))
    hs = f32(inputs["hidden_states"])
    mask = f32(inputs["attention_mask"])
    M, W1, b1, W2, b2 = (f32(inputs["M"]), f32(inputs["W_ct1"]),
                         f32(inputs["b_ct1"]), f32(inputs["W_ct2"]),
                         f32(inputs["b_ct2"]))
    R = ((M.T @ W1.T + b1).T @ (M @ W2.T + b2)).astype(np.float32)
    bias_scale = float(np.asarray(inputs["bias_scale"]).reshape(-1)[0])
    ln_g, ln_b = f32(inputs["ln_g"]), f32(inputs["ln_b"])

    def pack_f8(wT):
        # (H, J) fp8 -> (128, KO, J): partition-major
        w8 = np.ascontiguousarray(wT * WS).astype(F8NP)
        return np.ascontiguousarray(
            w8.reshape(KO, 128, w8.shape[1]).transpose(1, 0, 2))

    def pack_f8_pair(wT):
        # (H, J) fp8 -> (128, KO//2, J//128, 2, 128): contiguous k-pairs per
        # 128-wide output chunk (DoubleRow stationary layout)
        w = pack_f8(wT)                               # (128, KO, J)
        J = w.shape[2]
        return np.ascontiguousarray(
            w.reshape(128, KO // 2, 2, J // 128, 128).transpose(0, 1, 3, 2, 4))

    wf1 = f32(inputs["Wf1"]) * ln_g[None, :]          # fold LN gamma
    wf1T = np.ascontiguousarray(wf1.T).astype(BF16)   # (H, I)
    wf1p = np.ascontiguousarray(
        wf1T.reshape(KO, 128, IC // 2, 256).transpose(2, 0, 1, 3)
            .transpose(0, 2, 1, 3))                   # (IC//2, 128, KO, 256)
    wf2T = np.ascontiguousarray(f32(inputs["Wf2"]).T).astype(BF16)  # (I, H)
    wf2p = np.ascontiguousarray(wf2T.reshape(IC, 128, H))
    wptm8 = pack_f8(f32(inputs["W_ptm"]).T)           # (128, KO, P)
    wptm8p = np.zeros((128, KO, 64), F8NP)
    wptm8p[:, :, :P] = wptm8
    wptm8p = np.ascontiguousarray(
        wptm8p.reshape(128, KO // 2, 2, 64))
    shared = {
        "wqT": pack_f8_pair(f32(inputs["Wq"]).T),
        "wkT": pack_f8_pair(f32(inputs["Wk"]).T),
        "wvT": pack_f8(f32(inputs["Wv"]).T), "wptmT": wptm8p,
        "rmat": np.ascontiguousarray(R).astype(BF16),
        "wf1T": wf1p, "wf2T": wf2p,
        "bq": 0.125 * f32(inputs["bq"]).reshape(KO, 128),
        "bk": f32(inputs["bk"]).reshape(KO, 128),
        "bptm": f32(inputs["b_ptm"]).reshape(P, 1),
        "bf1": (f32(inputs["bf1"]) + f32(inputs["Wf1"]) @ ln_b).reshape(IC, 128),
        "lng": ln_g,
        "lnbf": ln_b + f32(inputs["bf2"]),
    }
    bv = f32(inputs["bv"])
    in_maps = []
    for c in range(8):
        b, half = c // 2, c % 2
        r0 = half * SQ
        hroll = np.roll(hs[b], -r0, axis=0)           # (S, H)
        hT8 = np.ascontiguousarray(hroll.T).astype(F8NP)   # (H, S)
        mb = np.roll((1.0 - mask[b]) * np.float32(-1e30), -r0)
        m = dict(shared)
        hTc = hT8.reshape(KO, 128, S).transpose(1, 0, 2)   # (128, KO, S)
        m["hT"] = np.ascontiguousarray(hTc)
        m["hTp"] = np.ascontiguousarray(
            hTc.reshape(128, KO // 2, 2, MC, 128).transpose(0, 1, 3, 2, 4))
        m["hres"] = np.ascontiguousarray(hs[b, r0:r0 + SQ] + bv[None, :])
        m["mb"] = np.ascontiguousarray(mb.reshape(MC, 128))
        in_maps.append(m)
    return in_maps, bias_scale


def kernel(**inputs) -> np.ndarray:
    in_maps, bias_scale = _prep_inputs(inputs)
    nc = _get_nc(bias_scale)
    res = run_bass_kernel_spmd(nc, in_maps, list(range(8)))
    out = np.zeros((B, S, H), np.float32)
    for c in range(8):
        b, half = c // 2, c % 2
        r0 = half * SQ
        out[b, r0:r0 + SQ] = res.results[c]["out"]
    return out
